# revision 8
# baseline (speedup 1.0000x reference)
"""Trainium2 Bass kernel for CustomizablePatchDominantGradientOrientation.

Pipeline per patch (32x32, fp32):
  sobel (replicate pad, [1,2,1]x[-1,0,1] separable; /8 dropped - the final
  angle is invariant to a global scale on (gx, gy, mag))
  mag = sqrt(gx^2+gy^2+eps'), theta = 2*atan(gy/(mag+gx))  (half-angle atan2)
  soft 36-bin histogram of theta weighted by mag  -> 36 fused custom-DVE
  passes (relu(min(a-c0, c1-a))*mag with free-axis accumulate)
  circular [w0,w1,w2] smoothing, argmax, parabolic refinement -> angle.

Engine split: the 36 histogram passes + magnitude/orientation customs run on
DVE (the bottleneck, ~1 elem/cycle fused mask+mul+reduce); the whole sobel
runs on the otherwise-idle Pool (GPSIMD) engine as add/sub tensor_tensor
ops; sqrt/arctan run on the Scalar (ACT) engine.

Data parallel: B=32768 patches sharded over 8 NeuronCores (4096 each);
per core 32 tiles of [128 patches x 1024 pixels].  Layout is patch-major:
partitions = patches, free axis = pixels.
"""

import math

import numpy as np

NBINS = 36
PI = math.pi
PATCH = 32
HW = PATCH * PATCH
P = 128          # partitions (patches per tile)
N_CORES = 8
GROUP = 4        # tiles per ACT-table-set phase group

_BUILD_CACHE = {}
_OPS_REGISTERED = {}


# --------------------------------------------------------------------------
# custom DVE ops
# --------------------------------------------------------------------------
def _register_custom_ops():
    """Register the fused ops at runtime (row assignment + sha pin, exactly
    what a source-level `OPS.append` would do)."""
    if _OPS_REGISTERED:
        return _OPS_REGISTERED
    from operator import add as _op_add

    import concourse.dve_ops as dve_ops
    from concourse.dve_ops import DveOp
    from concourse.dve_spec import (
        Spec, Src0, Src1, C0, C1, Zero, relu, minn, maxx, lower, _has_src1,
        sq,
    )
    from concourse.dve_uop import DveOpSpec

    def _reg(name, spec):
        if name in dve_ops._SUB_OPCODE_FOR_NAME:
            for op in dve_ops.OPS:
                if op.name == name:
                    return op
        row = dve_ops._CUSTOM_DVE_ROW_BASE + len(dve_ops.OPS)
        assert row < 0x20, "custom-DVE row budget exhausted"
        dve_ops._SUB_OPCODE_FOR_NAME[name] = row
        shas = {}
        for ver in ("v3", "v4"):
            s = DveOpSpec(name=name, opcode=row, uops=lower(spec, ver=ver),
                          rd1_en=_has_src1(spec))
            shas[ver] = s.sha(ver)
        op = DveOp(name, spec, subdim=False, uops_sha=shas)
        dve_ops.OPS.append(op)
        dve_ops.CUSTOM_DVE_SPECS[name] = spec
        return op

    def _tri_ref(in0, in1, s0, s1, imm2):
        b = np.maximum(np.minimum(in0 - s0, s1 - in0), 0.0).astype(np.float32) * in1
        return b, b.reshape(b.shape[0], -1).sum(axis=-1, keepdims=True)

    def _wrap_ref(in0, in1, s0, s1, imm2):
        b = np.maximum(np.maximum(s0 - in0, in0 - s1), 0.0).astype(np.float32) * in1
        return b, b.reshape(b.shape[0], -1).sum(axis=-1, keepdims=True)

    def _sqsum_ref(in0, in1, s0, s1, imm2):
        return (in0 * in0 + in1 * in1 + s0).astype(np.float32)

    def _rsqrt_nrm_ref(in0, in1, s0, s1, imm2):
        return (((s0 - in0 * in1 * in1 * s1) * in1) * in0).astype(np.float32)

    def _addmax_ref(in0, in1, s0, s1, imm2):
        return np.maximum(in0 + in1, s0).astype(np.float32)

    _OPS_REGISTERED["tri"] = _reg(
        "HIST_TRI_ANT",
        Spec(body=relu(minn(Src0 - C0, C1 - Src0)) * Src1,
             accum=_op_add, accum_init=Zero, reference=_tri_ref))
    _OPS_REGISTERED["wrap"] = _reg(
        "HIST_WRAP_ANT",
        Spec(body=relu(maxx(C0 - Src0, Src0 - C1)) * Src1,
             accum=_op_add, accum_init=Zero, reference=_wrap_ref))
    # g2 = gx^2 + gy^2 + c0  (exact fp32 squares on DVE)
    _OPS_REGISTERED["sqsum"] = _reg(
        "SQSUM_ANT",
        Spec(body=sq(Src0) + sq(Src1) + C0, reference=_sqsum_ref))
    # m = ((c0 - g2*rc^2*c1)*rc)*g2  (Newton step toward rsqrt, times g2)
    _OPS_REGISTERED["rsqrt_nrm"] = _reg(
        "RSQRT_NRM_ANT",
        Spec(body=((C0 - Src0 * sq(Src1) * C1) * Src1) * Src0,
             reference=_rsqrt_nrm_ref))
    _OPS_REGISTERED["addmax"] = _reg(
        "ADD_MAX_ANT",
        Spec(body=maxx(Src0 + Src1, C0), reference=_addmax_ref))
    return _OPS_REGISTERED


# --------------------------------------------------------------------------
# kernel build
# --------------------------------------------------------------------------
def _build(b_core, smooth_w, wk_is_ones):
    import concourse.bacc as bacc
    import concourse.mybir as mybir
    from concourse.tile import TileContext
    from concourse.bass import broadcast_tensor_aps

    ops = _register_custom_ops()
    TRI, WRAP = ops["tri"], ops["wrap"]
    SQSUM, RSQRT_NRM, ADDMAX = ops["sqsum"], ops["rsqrt_nrm"], ops["addmax"]

    f32 = mybir.dt.float32
    f16 = mybir.dt.float16
    Alu = mybir.AluOpType
    Act = mybir.ActivationFunctionType

    n_tiles = b_core // P
    assert b_core % P == 0
    w0, w1, w2 = (float(x) for x in smooth_w)

    nc = bacc.Bacc(None, target_bir_lowering=False, debug=False)
    patch_in = nc.dram_tensor("patch", [b_core, HW], f32, kind="ExternalInput")
    # consts: iota36 repeated n_tiles times, then (iota36 - 64) repeated
    consts_in = nc.dram_tensor("consts", [P, 2 * n_tiles * NBINS], f32,
                               kind="ExternalInput")
    wk_in = None
    if not wk_is_ones:
        wk_in = nc.dram_tensor("wk", [P, HW], f32, kind="ExternalInput")
    out_t = nc.dram_tensor("angle", [b_core], f32, kind="ExternalOutput")

    # per-bin tri constants in atan units: t = (36/pi)*a + 18
    # bin k (k>=1): c0=(k-19)*pi/36, c1=(k-17)*pi/36
    # bin 0 wrap:   relu(max(c0 - a, a - c1)) with c0=-17pi/36, c1=17pi/36
    def c_lo(k):
        return (k - 19.0) * PI / 36.0

    def c_hi(k):
        return (k - 17.0) * PI / 36.0

    with TileContext(nc) as tc:
        with tc.tile_pool(name="pool", bufs=2) as pool, \
             tc.tile_pool(name="persist", bufs=1) as pp:
            IOTA = pp.tile([P, n_tiles, NBINS], f32)
            IOTA64 = pp.tile([P, n_tiles, NBINS], f32)
            nc.sync.dma_start(IOTA[:], consts_in[:, 0:n_tiles * NBINS])
            nc.sync.dma_start(IOTA64[:], consts_in[:, n_tiles * NBINS:])
            WK = None
            if wk_in is not None:
                WK = pp.tile([P, HW], f32)
                nc.sync.dma_start(WK[:], wk_in[:])

            HEXT = pp.tile([P, n_tiles, NBINS + 2], f32)
            ANG = pp.tile([P, n_tiles], f32)
            out_view = out_t[:].rearrange("(t p) -> p t", p=P)

            def tail(tiles):
                """smoothing, argmax, parabolic refinement for a tile range
                (all [P, len(tiles), ...] slices of the persistent tensors)."""
                ts = slice(tiles.start, tiles.stop)
                HX = HEXT[:, ts, :]
                nc.vector.tensor_copy(HX[:, :, 0:1], HX[:, :, 36:37])
                nc.vector.tensor_copy(HX[:, :, 37:38], HX[:, :, 1:2])

                SM = pool.tile([P, len(range(tiles.start, tiles.stop)),
                                NBINS], f32, tag="t_sm",
                               name=f"sm{tiles.start}")
                nc.vector.tensor_scalar(SM[:], HX[:, :, 2:38], w2, None,
                                        Alu.mult)
                nc.vector.scalar_tensor_tensor(
                    out=SM[:], in0=HX[:, :, 0:36], scalar=w0, in1=SM[:],
                    op0=Alu.mult, op1=Alu.add)
                HS = pool.tile([P, SM.shape[1], NBINS], f32, tag="t_hs",
                               name=f"hs{tiles.start}")
                nc.vector.scalar_tensor_tensor(
                    out=HS[:], in0=HX[:, :, 1:37], scalar=w1, in1=SM[:],
                    op0=Alu.mult, op1=Alu.add)

                IOT = IOTA[:, ts, :]
                IOT64 = IOTA64[:, ts, :]
                VMAX = pool.tile([P, SM.shape[1], 1], f32, tag="t_vm",
                                 name=f"vm{tiles.start}")
                nc.vector.tensor_reduce(VMAX[:], HS[:], mybir.AxisListType.X,
                                        Alu.max)
                EQ = pool.tile([P, SM.shape[1], NBINS], f32, tag="t_eq",
                               name=f"eq{tiles.start}")
                hs_b, vmax_b = broadcast_tensor_aps(HS[:], VMAX[:])
                nc.vector.tensor_tensor(EQ[:], hs_b, vmax_b, Alu.is_equal)
                nc.vector.tensor_tensor(EQ[:], EQ[:], IOT64, Alu.mult)
                IDX = pool.tile([P, SM.shape[1], 1], f32, tag="t_ix",
                                name=f"ix{tiles.start}")
                nc.vector.tensor_reduce(IDX[:], EQ[:], mybir.AxisListType.X,
                                        Alu.min)
                nc.vector.tensor_scalar(IDX[:], IDX[:], 64.0, None, Alu.add)

                def neighbor_value(shift, wrap_thr, wrap_add, nm):
                    IDXN = pool.tile([P, SM.shape[1], 1], f32,
                                     tag=f"t_in{nm}",
                                     name=f"idxn_{nm}{tiles.start}")
                    nc.vector.tensor_scalar(IDXN[:], IDX[:], float(shift),
                                            None, Alu.add)
                    WADJ = pool.tile([P, SM.shape[1], 1], f32,
                                     tag=f"t_wa{nm}",
                                     name=f"wadj_{nm}{tiles.start}")
                    if wrap_add < 0:
                        nc.vector.tensor_scalar(WADJ[:], IDXN[:], wrap_thr,
                                                float(wrap_add), Alu.is_gt,
                                                Alu.mult)
                    else:
                        nc.vector.tensor_scalar(WADJ[:], IDXN[:], wrap_thr,
                                                float(wrap_add), Alu.is_lt,
                                                Alu.mult)
                    nc.vector.tensor_tensor(IDXN[:], IDXN[:], WADJ[:],
                                            Alu.add)
                    DIF = pool.tile([P, SM.shape[1], NBINS], f32,
                                    tag=f"t_df{nm}",
                                    name=f"dif_{nm}{tiles.start}")
                    iota_b, idxn_b = broadcast_tensor_aps(IOT, IDXN[:])
                    nc.vector.tensor_tensor(DIF[:], iota_b, idxn_b,
                                            Alu.subtract)
                    nc.vector.tensor_scalar(DIF[:], DIF[:], 0.0, None,
                                            Alu.is_equal)
                    nc.vector.tensor_tensor(DIF[:], DIF[:], HS[:], Alu.mult)
                    V = pool.tile([P, SM.shape[1], 1], f32, tag=f"t_v{nm}",
                                  name=f"v_{nm}{tiles.start}")
                    nc.vector.tensor_reduce(V[:], DIF[:],
                                            mybir.AxisListType.X, Alu.add)
                    return V

                VP = neighbor_value(+1, 35.5, -36.0, "p")
                VM = neighbor_value(-1, -0.5, +36.0, "m")

                NUM = pool.tile([P, SM.shape[1], 1], f32, tag="t_nu",
                                name=f"nu{tiles.start}")
                nc.vector.tensor_tensor(NUM[:], VP[:], VM[:], Alu.subtract)
                SUMN = pool.tile([P, SM.shape[1], 1], f32, tag="t_su",
                                 name=f"su{tiles.start}")
                nc.vector.tensor_tensor(SUMN[:], VP[:], VM[:], Alu.add)
                DEN = pool.tile([P, SM.shape[1], 1], f32, tag="t_de",
                                name=f"de{tiles.start}")
                nc.vector.tensor_scalar(DEN[:], VMAX[:], 2.0, None, Alu.mult)
                nc.vector.tensor_tensor(DEN[:], DEN[:], SUMN[:], Alu.subtract)
                RECD = pool.tile([P, SM.shape[1], 1], f32, tag="t_rd",
                                 name=f"rd{tiles.start}")
                SCD = pool.tile([P, SM.shape[1], 1], f32, tag="t_sc",
                                name=f"sc{tiles.start}")
                nc.vector.reciprocal_approx_accurate(RECD[:], DEN[:], SCD[:])
                REF = pool.tile([P, SM.shape[1], 1], f32, tag="t_rf",
                                name=f"rf{tiles.start}")
                nc.vector.scalar_tensor_tensor(
                    out=REF[:], in0=NUM[:], scalar=0.5, in1=RECD[:],
                    op0=Alu.mult, op1=Alu.mult)
                nc.vector.tensor_tensor(REF[:], IDX[:], REF[:], Alu.add)
                nc.vector.tensor_scalar(ANG[:, ts], REF[:, :, 0],
                                        -2.0 * PI / NBINS, PI, Alu.mult,
                                        Alu.add)
                nc.sync.dma_start(out_view[:, ts], ANG[:, ts])

            n_groups = (n_tiles + GROUP - 1) // GROUP
            for g in range(n_groups):
                tiles = range(g * GROUP, min((g + 1) * GROUP, n_tiles))
                slot = {}
                # ---- phase A: sobel (Pool), magnitude (DVE + sqrt table) --
                for t in tiles:
                    s = t % GROUP
                    X = pool.tile([P, HW], f32, tag="x", bufs=3, name=f"x{t}")
                    nc.sync.dma_start(X[:], patch_in[t * P:(t + 1) * P, :])
                    X3 = X.rearrange("p (r c) -> p r c", c=PATCH)

                    # vertical [1,2,1] via pair-sums: A[i] = X[i] + X[i+32]
                    AV = pool.tile([P, 992], f32, tag="av", name=f"av{t}")
                    nc.gpsimd.tensor_tensor(AV[:], X[:, 0:992], X[:, 32:1024],
                                            Alu.add)
                    SV = pool.tile([P, HW], f32, tag="sv", name=f"sv{t}")
                    nc.gpsimd.tensor_tensor(SV[:, 32:992], AV[:, 0:960],
                                            AV[:, 32:992], Alu.add)
                    E0 = pool.tile([P, 32], f32, tag="e0", name=f"e0{t}")
                    nc.gpsimd.tensor_tensor(E0[:], X[:, 0:32], X[:, 0:32],
                                            Alu.add)
                    nc.gpsimd.tensor_tensor(SV[:, 0:32], AV[:, 0:32], E0[:],
                                            Alu.add)
                    E1 = pool.tile([P, 32], f32, tag="e1", name=f"e1{t}")
                    nc.gpsimd.tensor_tensor(E1[:], X[:, 992:1024],
                                            X[:, 992:1024], Alu.add)
                    nc.gpsimd.tensor_tensor(SV[:, 992:1024], AV[:, 960:992],
                                            E1[:], Alu.add)
                    SV3 = SV.rearrange("p (r c) -> p r c", c=PATCH)

                    GX = pool.tile([P, HW], f32, tag=f"gx{s}", bufs=1,
                                   name=f"gx{t}")
                    GX3 = GX.rearrange("p (r c) -> p r c", c=PATCH)
                    nc.gpsimd.tensor_tensor(
                        GX3[:, :, 1:31], SV3[:, :, 2:32], SV3[:, :, 0:30],
                        Alu.subtract)
                    nc.gpsimd.tensor_tensor(
                        GX3[:, :, 0:1], SV3[:, :, 1:2], SV3[:, :, 0:1],
                        Alu.subtract)
                    nc.gpsimd.tensor_tensor(
                        GX3[:, :, 31:32], SV3[:, :, 31:32], SV3[:, :, 30:31],
                        Alu.subtract)

                    # horizontal [1,2,1] via pair-sums within rows
                    BH = pool.tile([P, 32, 31], f32, tag="bh", name=f"bh{t}")
                    nc.gpsimd.tensor_tensor(BH[:], X3[:, :, 0:31],
                                            X3[:, :, 1:32], Alu.add)
                    SH = pool.tile([P, HW], f32, tag="sh", name=f"sh{t}")
                    SH3 = SH.rearrange("p (r c) -> p r c", c=PATCH)
                    nc.gpsimd.tensor_tensor(SH3[:, :, 1:31], BH[:, :, 0:30],
                                            BH[:, :, 1:31], Alu.add)
                    E2 = pool.tile([P, 32, 1], f32, tag="e2", name=f"e2{t}")
                    nc.gpsimd.tensor_tensor(E2[:], X3[:, :, 0:1],
                                            X3[:, :, 0:1], Alu.add)
                    nc.gpsimd.tensor_tensor(SH3[:, :, 0:1], BH[:, :, 0:1],
                                            E2[:], Alu.add)
                    E3 = pool.tile([P, 32, 1], f32, tag="e3", name=f"e3{t}")
                    nc.gpsimd.tensor_tensor(E3[:], X3[:, :, 31:32],
                                            X3[:, :, 31:32], Alu.add)
                    nc.gpsimd.tensor_tensor(SH3[:, :, 31:32], BH[:, :, 30:31],
                                            E3[:], Alu.add)

                    GY = pool.tile([P, HW], f32, tag=f"gy{s}", bufs=1,
                                   name=f"gy{t}")
                    nc.gpsimd.tensor_tensor(
                        GY[:, 32:992], SH[:, 64:1024], SH[:, 0:960],
                        Alu.subtract)
                    nc.gpsimd.tensor_tensor(
                        GY[:, 0:32], SH[:, 32:64], SH[:, 0:32], Alu.subtract)
                    nc.gpsimd.tensor_tensor(
                        GY[:, 992:1024], SH[:, 992:1024], SH[:, 960:992],
                        Alu.subtract)

                    if WK is not None:
                        nc.gpsimd.tensor_tensor(GX[:], GX[:], WK[:], Alu.mult)
                        nc.gpsimd.tensor_tensor(GY[:], GY[:], WK[:], Alu.mult)
                    slot[t] = [GX, GY]

                # ---- magnitude chain, op-major across the group for
                # pipeline depth (no DVE stalls on ACT/Pool latency) ----
                g2s, m0s, ms = {}, {}, {}
                for t in tiles:
                    s = t % GROUP
                    GX, GY = slot[t][0], slot[t][1]
                    # g2 = gx^2 + gy^2 + eps  (eps scaled by 8^2 vs reference)
                    G2 = pool.tile([P, HW], f32, tag=f"g2a{s}", bufs=1,
                                   name=f"g2{t}")
                    nc.vector._custom_dve(SQSUM, out=G2[:], in0=GX[:],
                                          in1=GY[:], s0=6.4e-17)
                    g2s[t] = G2
                for t in tiles:
                    s = t % GROUP
                    M0 = pool.tile([P, HW], f32, tag=f"msq{s}", bufs=1,
                                   name=f"msq{t}")
                    nc.scalar.activation(M0[:], g2s[t][:], Act.Sqrt)
                    m0s[t] = M0
                for t in tiles:
                    s = t % GROUP
                    # one Newton step: m = (1.5 - g2*rc^2*0.5)*rc*g2
                    RC = pool.tile([P, HW], f32, tag="rcf", name=f"rc{t}")
                    nc.vector.reciprocal_approx_fast(RC[:], m0s[t][:])
                    M = pool.tile([P, HW], f32, tag=f"m{s}", bufs=1,
                                  name=f"m{t}")
                    nc.vector._custom_dve(RSQRT_NRM, out=M[:], in0=g2s[t][:],
                                          in1=RC[:], s0=1.5, s1=0.5)
                    ms[t] = M

                # ---- phase B: orientation (op-major) + histogram ----
                ds, qs, a16s = {}, {}, {}
                for t in tiles:
                    s = t % GROUP
                    # d = max(m + gx, 1e-30): the clamp both avoids the
                    # recip(0)=NaN edge and pins rounding-negative d to the
                    # correct wrap side (t -> 36/0 by sign of gy).
                    D = pool.tile([P, HW], f32, tag=f"g2a{s}", bufs=1,
                                  name=f"d{t}")
                    nc.vector._custom_dve(ADDMAX, out=D[:], in0=ms[t][:],
                                          in1=slot[t][0][:], s0=1e-30)
                    ds[t] = D
                for t in tiles:
                    s = t % GROUP
                    RC = pool.tile([P, HW], f32, tag="rcf", name=f"rcb{t}")
                    SC = pool.tile([P, HW], f32, tag="scf", name=f"scb{t}")
                    nc.vector.reciprocal_approx_accurate(RC[:], ds[t][:],
                                                         SC[:])
                    Q = pool.tile([P, HW], f32, tag=f"msq{s}", bufs=1,
                                  name=f"q{t}")
                    nc.vector.scalar_tensor_tensor(
                        out=Q[:], in0=slot[t][1][:], scalar=1.0, in1=RC[:],
                        op0=Alu.mult, op1=Alu.mult)
                    qs[t] = Q
                for t in tiles:
                    s = t % GROUP
                    A = pool.tile([P, HW], f32, tag=f"a{s % 2}", bufs=2,
                                  name=f"a{t}")
                    nc.scalar.activation(A[:], qs[t][:], Act.Arctan)
                    a16s[t] = A

                for t in tiles:
                    A, M16 = a16s[t], ms[t]
                    SCR = pool.tile([P, HW], f16, tag="scr", name=f"scr{t}")
                    for k in range(NBINS):
                        acc = HEXT[:, t, k + 1:k + 2]
                        if k == 0:
                            nc.vector._custom_dve(
                                WRAP, out=SCR[:], in0=A[:], in1=M16[:],
                                s0=-17.0 * PI / 36.0, s1=17.0 * PI / 36.0,
                                accum_out=acc)
                        else:
                            nc.vector._custom_dve(
                                TRI, out=SCR[:], in0=A[:], in1=M16[:],
                                s0=c_lo(k), s1=c_hi(k), accum_out=acc)

                # ---- tail for the previous group (overlaps next group) ----
                if g > 0:
                    tail(range((g - 1) * GROUP, g * GROUP))
            tail(range((n_groups - 1) * GROUP, n_tiles))

    nc.compile()
    return nc


def _get_built(b_core, smooth_w, wk_is_ones):
    key = (b_core, tuple(float(x) for x in smooth_w), bool(wk_is_ones))
    if key not in _BUILD_CACHE:
        _BUILD_CACHE[key] = _build(b_core, smooth_w, wk_is_ones)
    return _BUILD_CACHE[key]


# --------------------------------------------------------------------------
# host entry point
# --------------------------------------------------------------------------
def kernel(patch, weight_kernel, smooth_w):
    from concourse import bass_utils

    patch = np.ascontiguousarray(np.asarray(patch, dtype=np.float32))
    weight_kernel = np.asarray(weight_kernel, dtype=np.float32)
    smooth_w = np.asarray(smooth_w, dtype=np.float32)

    B = patch.shape[0]
    assert B % (N_CORES * P) == 0, f"B={B} not divisible by {N_CORES * P}"
    b_core = B // N_CORES
    n_tiles = b_core // P

    wk_is_ones = bool(np.all(weight_kernel == 1.0))
    nc = _get_built(b_core, smooth_w, wk_is_ones)

    x = patch.reshape(N_CORES, b_core, HW)

    iota = np.tile(np.arange(NBINS, dtype=np.float32), n_tiles)
    consts_row = np.concatenate([iota, iota - 64.0]).astype(np.float32)
    consts = np.ascontiguousarray(
        np.broadcast_to(consts_row, (P, consts_row.size)))

    in_maps = []
    for i in range(N_CORES):
        m = {"patch": np.ascontiguousarray(x[i]), "consts": consts}
        if not wk_is_ones:
            m["wk"] = np.ascontiguousarray(
                np.broadcast_to(weight_kernel.reshape(-1), (P, HW)))
        in_maps.append(m)

    res = bass_utils.run_bass_kernel_spmd(nc, in_maps,
                                          core_ids=list(range(N_CORES)))
    out = np.concatenate([r["angle"] for r in res.results])
    return out.astype(np.float32)


# revision 9
# speedup vs baseline: 1.1817x; 1.1817x over previous
"""Trainium2 Bass kernel for CustomizablePatchDominantGradientOrientation.

Pipeline per patch (32x32, fp32):
  sobel (replicate pad, [1,2,1]x[-1,0,1] separable; /8 dropped - the final
  angle is invariant to a global scale on (gx, gy, mag))
  mag = sqrt(gx^2+gy^2+eps'), theta = 2*atan(gy/(mag+gx))  (half-angle atan2)
  soft 36-bin histogram of theta weighted by mag  -> 36 fused custom-DVE
  passes (relu(min(a-c0, c1-a))*mag with free-axis accumulate)
  circular [w0,w1,w2] smoothing, argmax, parabolic refinement -> angle.

Engine split: the 36 histogram passes + magnitude/orientation customs run on
DVE (the bottleneck, ~1 elem/cycle fused mask+mul+reduce); the whole sobel
runs on the otherwise-idle Pool (GPSIMD) engine as add/sub tensor_tensor
ops; sqrt/arctan run on the Scalar (ACT) engine.

Data parallel: B=32768 patches sharded over 8 NeuronCores (4096 each);
per core 32 tiles of [128 patches x 1024 pixels].  Layout is patch-major:
partitions = patches, free axis = pixels.
"""

import math

import numpy as np

NBINS = 36
PI = math.pi
PATCH = 32
HW = PATCH * PATCH
P = 128          # partitions (patches per tile)
N_CORES = 8
GROUP = 4        # tiles per ACT-table-set phase group

_BUILD_CACHE = {}
_OPS_REGISTERED = {}


# --------------------------------------------------------------------------
# custom DVE ops
# --------------------------------------------------------------------------
def _register_custom_ops():
    """Register the fused ops at runtime (row assignment + sha pin, exactly
    what a source-level `OPS.append` would do)."""
    if _OPS_REGISTERED:
        return _OPS_REGISTERED
    from operator import add as _op_add

    import concourse.dve_ops as dve_ops
    from concourse.dve_ops import DveOp
    from concourse.dve_spec import (
        Spec, Src0, Src1, C0, C1, Zero, relu, minn, maxx, lower, _has_src1,
        sq,
    )
    from concourse.dve_uop import DveOpSpec

    def _reg(name, spec):
        if name in dve_ops._SUB_OPCODE_FOR_NAME:
            for op in dve_ops.OPS:
                if op.name == name:
                    return op
        row = dve_ops._CUSTOM_DVE_ROW_BASE + len(dve_ops.OPS)
        assert row < 0x20, "custom-DVE row budget exhausted"
        dve_ops._SUB_OPCODE_FOR_NAME[name] = row
        shas = {}
        for ver in ("v3", "v4"):
            s = DveOpSpec(name=name, opcode=row, uops=lower(spec, ver=ver),
                          rd1_en=_has_src1(spec))
            shas[ver] = s.sha(ver)
        op = DveOp(name, spec, subdim=False, uops_sha=shas)
        dve_ops.OPS.append(op)
        dve_ops.CUSTOM_DVE_SPECS[name] = spec
        return op

    def _tri_ref(in0, in1, s0, s1, imm2):
        b = np.maximum(np.minimum(in0 - s0, s1 - in0), 0.0).astype(np.float32) * in1
        return b, b.reshape(b.shape[0], -1).sum(axis=-1, keepdims=True)

    def _wrap_ref(in0, in1, s0, s1, imm2):
        b = np.maximum(np.maximum(s0 - in0, in0 - s1), 0.0).astype(np.float32) * in1
        return b, b.reshape(b.shape[0], -1).sum(axis=-1, keepdims=True)

    def _sqsum_ref(in0, in1, s0, s1, imm2):
        return (in0 * in0 + in1 * in1 + s0).astype(np.float32)

    def _rsqrt_nrm_ref(in0, in1, s0, s1, imm2):
        return (((s0 - in0 * in1 * in1 * s1) * in1) * in0).astype(np.float32)

    def _addmax_ref(in0, in1, s0, s1, imm2):
        return np.maximum(in0 + in1, s0).astype(np.float32)

    _OPS_REGISTERED["tri"] = _reg(
        "HIST_TRI_ANT",
        Spec(body=relu(minn(Src0 - C0, C1 - Src0)) * Src1,
             accum=_op_add, accum_init=Zero, reference=_tri_ref))
    _OPS_REGISTERED["wrap"] = _reg(
        "HIST_WRAP_ANT",
        Spec(body=relu(maxx(C0 - Src0, Src0 - C1)) * Src1,
             accum=_op_add, accum_init=Zero, reference=_wrap_ref))
    # g2 = gx^2 + gy^2 + c0  (exact fp32 squares on DVE)
    _OPS_REGISTERED["sqsum"] = _reg(
        "SQSUM_ANT",
        Spec(body=sq(Src0) + sq(Src1) + C0, reference=_sqsum_ref))
    # m = ((c0 - g2*rc^2*c1)*rc)*g2  (Newton step toward rsqrt, times g2)
    _OPS_REGISTERED["rsqrt_nrm"] = _reg(
        "RSQRT_NRM_ANT",
        Spec(body=((C0 - Src0 * sq(Src1) * C1) * Src1) * Src0,
             reference=_rsqrt_nrm_ref))
    _OPS_REGISTERED["addmax"] = _reg(
        "ADD_MAX_ANT",
        Spec(body=maxx(Src0 + Src1, C0), reference=_addmax_ref))
    return _OPS_REGISTERED


# --------------------------------------------------------------------------
# kernel build
# --------------------------------------------------------------------------
def _build(b_core, smooth_w, wk_is_ones):
    import concourse.bacc as bacc
    import concourse.mybir as mybir
    from concourse.tile import TileContext
    from concourse.bass import broadcast_tensor_aps

    ops = _register_custom_ops()
    TRI, WRAP = ops["tri"], ops["wrap"]
    SQSUM, RSQRT_NRM, ADDMAX = ops["sqsum"], ops["rsqrt_nrm"], ops["addmax"]

    f32 = mybir.dt.float32
    f16 = mybir.dt.float16
    Alu = mybir.AluOpType
    Act = mybir.ActivationFunctionType

    n_tiles = b_core // P
    assert b_core % P == 0
    w0, w1, w2 = (float(x) for x in smooth_w)

    nc = bacc.Bacc(None, target_bir_lowering=False, debug=False)
    patch_in = nc.dram_tensor("patch", [b_core, HW], f32, kind="ExternalInput")
    # consts: iota36 repeated n_tiles times, then (iota36 - 64) repeated
    consts_in = nc.dram_tensor("consts", [P, 2 * n_tiles * NBINS], f32,
                               kind="ExternalInput")
    wk_in = None
    if not wk_is_ones:
        wk_in = nc.dram_tensor("wk", [P, HW], f32, kind="ExternalInput")
    out_t = nc.dram_tensor("angle", [b_core], f32, kind="ExternalOutput")

    # per-bin tri constants in atan units: t = (36/pi)*a + 18
    # bin k (k>=1): c0=(k-19)*pi/36, c1=(k-17)*pi/36
    # bin 0 wrap:   relu(max(c0 - a, a - c1)) with c0=-17pi/36, c1=17pi/36
    def c_lo(k):
        return (k - 19.0) * PI / 36.0

    def c_hi(k):
        return (k - 17.0) * PI / 36.0

    with TileContext(nc) as tc:
        with tc.tile_pool(name="pool", bufs=2) as pool, \
             tc.tile_pool(name="persist", bufs=1) as pp, \
             tc.psum_pool(name="psum", bufs=2) as psp:
            IOTA = pp.tile([P, n_tiles, NBINS], f32)
            IOTA64 = pp.tile([P, n_tiles, NBINS], f32)
            nc.sync.dma_start(IOTA[:], consts_in[:, 0:n_tiles * NBINS])
            nc.sync.dma_start(IOTA64[:], consts_in[:, n_tiles * NBINS:])
            WK = None
            if wk_in is not None:
                WK = pp.tile([P, HW], f32)
                nc.sync.dma_start(WK[:], wk_in[:])

            HEXT = pp.tile([P, n_tiles, NBINS + 2], f32)
            ANG = pp.tile([P, n_tiles], f32)
            out_view = out_t[:].rearrange("(t p) -> p t", p=P)

            def tail(tiles):
                """smoothing, argmax, parabolic refinement for a tile range
                (all [P, len(tiles), ...] slices of the persistent tensors)."""
                ts = slice(tiles.start, tiles.stop)
                HX = HEXT[:, ts, :]
                nc.vector.tensor_copy(HX[:, :, 0:1], HX[:, :, 36:37])
                nc.vector.tensor_copy(HX[:, :, 37:38], HX[:, :, 1:2])

                SM = pool.tile([P, len(range(tiles.start, tiles.stop)),
                                NBINS], f32, tag="t_sm",
                               name=f"sm{tiles.start}")
                nc.vector.tensor_scalar(SM[:], HX[:, :, 2:38], w2, None,
                                        Alu.mult)
                nc.vector.scalar_tensor_tensor(
                    out=SM[:], in0=HX[:, :, 0:36], scalar=w0, in1=SM[:],
                    op0=Alu.mult, op1=Alu.add)
                HS = pool.tile([P, SM.shape[1], NBINS], f32, tag="t_hs",
                               name=f"hs{tiles.start}")
                nc.vector.scalar_tensor_tensor(
                    out=HS[:], in0=HX[:, :, 1:37], scalar=w1, in1=SM[:],
                    op0=Alu.mult, op1=Alu.add)

                IOT = IOTA[:, ts, :]
                IOT64 = IOTA64[:, ts, :]
                VMAX = pool.tile([P, SM.shape[1], 1], f32, tag="t_vm",
                                 name=f"vm{tiles.start}")
                nc.vector.tensor_reduce(VMAX[:], HS[:], mybir.AxisListType.X,
                                        Alu.max)
                EQ = pool.tile([P, SM.shape[1], NBINS], f32, tag="t_eq",
                               name=f"eq{tiles.start}")
                hs_b, vmax_b = broadcast_tensor_aps(HS[:], VMAX[:])
                nc.vector.tensor_tensor(EQ[:], hs_b, vmax_b, Alu.is_equal)
                nc.vector.tensor_tensor(EQ[:], EQ[:], IOT64, Alu.mult)
                IDX = pool.tile([P, SM.shape[1], 1], f32, tag="t_ix",
                                name=f"ix{tiles.start}")
                nc.vector.tensor_reduce(IDX[:], EQ[:], mybir.AxisListType.X,
                                        Alu.min)
                nc.vector.tensor_scalar(IDX[:], IDX[:], 64.0, None, Alu.add)

                def neighbor_value(shift, wrap_thr, wrap_add, nm):
                    IDXN = pool.tile([P, SM.shape[1], 1], f32,
                                     tag=f"t_in{nm}",
                                     name=f"idxn_{nm}{tiles.start}")
                    nc.vector.tensor_scalar(IDXN[:], IDX[:], float(shift),
                                            None, Alu.add)
                    WADJ = pool.tile([P, SM.shape[1], 1], f32,
                                     tag=f"t_wa{nm}",
                                     name=f"wadj_{nm}{tiles.start}")
                    if wrap_add < 0:
                        nc.vector.tensor_scalar(WADJ[:], IDXN[:], wrap_thr,
                                                float(wrap_add), Alu.is_gt,
                                                Alu.mult)
                    else:
                        nc.vector.tensor_scalar(WADJ[:], IDXN[:], wrap_thr,
                                                float(wrap_add), Alu.is_lt,
                                                Alu.mult)
                    nc.vector.tensor_tensor(IDXN[:], IDXN[:], WADJ[:],
                                            Alu.add)
                    DIF = pool.tile([P, SM.shape[1], NBINS], f32,
                                    tag=f"t_df{nm}",
                                    name=f"dif_{nm}{tiles.start}")
                    iota_b, idxn_b = broadcast_tensor_aps(IOT, IDXN[:])
                    nc.vector.tensor_tensor(DIF[:], iota_b, idxn_b,
                                            Alu.subtract)
                    nc.vector.tensor_scalar(DIF[:], DIF[:], 0.0, None,
                                            Alu.is_equal)
                    nc.vector.tensor_tensor(DIF[:], DIF[:], HS[:], Alu.mult)
                    V = pool.tile([P, SM.shape[1], 1], f32, tag=f"t_v{nm}",
                                  name=f"v_{nm}{tiles.start}")
                    nc.vector.tensor_reduce(V[:], DIF[:],
                                            mybir.AxisListType.X, Alu.add)
                    return V

                VP = neighbor_value(+1, 35.5, -36.0, "p")
                VM = neighbor_value(-1, -0.5, +36.0, "m")

                NUM = pool.tile([P, SM.shape[1], 1], f32, tag="t_nu",
                                name=f"nu{tiles.start}")
                nc.vector.tensor_tensor(NUM[:], VP[:], VM[:], Alu.subtract)
                SUMN = pool.tile([P, SM.shape[1], 1], f32, tag="t_su",
                                 name=f"su{tiles.start}")
                nc.vector.tensor_tensor(SUMN[:], VP[:], VM[:], Alu.add)
                DEN = pool.tile([P, SM.shape[1], 1], f32, tag="t_de",
                                name=f"de{tiles.start}")
                nc.vector.tensor_scalar(DEN[:], VMAX[:], 2.0, None, Alu.mult)
                nc.vector.tensor_tensor(DEN[:], DEN[:], SUMN[:], Alu.subtract)
                RECD = pool.tile([P, SM.shape[1], 1], f32, tag="t_rd",
                                 name=f"rd{tiles.start}")
                SCD = pool.tile([P, SM.shape[1], 1], f32, tag="t_sc",
                                name=f"sc{tiles.start}")
                nc.vector.reciprocal_approx_accurate(RECD[:], DEN[:], SCD[:])
                REF = pool.tile([P, SM.shape[1], 1], f32, tag="t_rf",
                                name=f"rf{tiles.start}")
                nc.vector.scalar_tensor_tensor(
                    out=REF[:], in0=NUM[:], scalar=0.5, in1=RECD[:],
                    op0=Alu.mult, op1=Alu.mult)
                nc.vector.tensor_tensor(REF[:], IDX[:], REF[:], Alu.add)
                nc.vector.tensor_scalar(ANG[:, ts], REF[:, :, 0],
                                        -2.0 * PI / NBINS, PI, Alu.mult,
                                        Alu.add)
                nc.sync.dma_start(out_view[:, ts], ANG[:, ts])

            n_groups = (n_tiles + GROUP - 1) // GROUP
            for g in range(n_groups):
                tiles = range(g * GROUP, min((g + 1) * GROUP, n_tiles))
                slot = {}
                # ---- phase A: sobel (Pool), magnitude (DVE + sqrt table) --
                for t in tiles:
                    s = t % GROUP
                    X = pool.tile([P, HW], f32, tag="x", bufs=3, name=f"x{t}")
                    nc.sync.dma_start(X[:], patch_in[t * P:(t + 1) * P, :])
                    X3 = X.rearrange("p (r c) -> p r c", c=PATCH)

                    # vertical [1,2,1] via pair-sums: A[i] = X[i] + X[i+32]
                    AV = pool.tile([P, 992], f32, tag="av", name=f"av{t}")
                    nc.gpsimd.tensor_tensor(AV[:], X[:, 0:992], X[:, 32:1024],
                                            Alu.add)
                    SV = pool.tile([P, HW], f32, tag="sv", name=f"sv{t}")
                    nc.gpsimd.tensor_tensor(SV[:, 32:992], AV[:, 0:960],
                                            AV[:, 32:992], Alu.add)
                    E0 = pool.tile([P, 32], f32, tag="e0", name=f"e0{t}")
                    nc.gpsimd.tensor_tensor(E0[:], X[:, 0:32], X[:, 0:32],
                                            Alu.add)
                    nc.gpsimd.tensor_tensor(SV[:, 0:32], AV[:, 0:32], E0[:],
                                            Alu.add)
                    E1 = pool.tile([P, 32], f32, tag="e1", name=f"e1{t}")
                    nc.gpsimd.tensor_tensor(E1[:], X[:, 992:1024],
                                            X[:, 992:1024], Alu.add)
                    nc.gpsimd.tensor_tensor(SV[:, 992:1024], AV[:, 960:992],
                                            E1[:], Alu.add)
                    SV3 = SV.rearrange("p (r c) -> p r c", c=PATCH)

                    GX = pool.tile([P, HW], f32, tag=f"gx{s}", bufs=1,
                                   name=f"gx{t}")
                    GX3 = GX.rearrange("p (r c) -> p r c", c=PATCH)
                    nc.gpsimd.tensor_tensor(
                        GX3[:, :, 1:31], SV3[:, :, 2:32], SV3[:, :, 0:30],
                        Alu.subtract)
                    nc.gpsimd.tensor_tensor(
                        GX3[:, :, 0:1], SV3[:, :, 1:2], SV3[:, :, 0:1],
                        Alu.subtract)
                    nc.gpsimd.tensor_tensor(
                        GX3[:, :, 31:32], SV3[:, :, 31:32], SV3[:, :, 30:31],
                        Alu.subtract)

                    # horizontal [1,2,1] via pair-sums within rows
                    BH = pool.tile([P, 32, 31], f32, tag="bh", name=f"bh{t}")
                    nc.gpsimd.tensor_tensor(BH[:], X3[:, :, 0:31],
                                            X3[:, :, 1:32], Alu.add)
                    SH = pool.tile([P, HW], f32, tag="sh", name=f"sh{t}")
                    SH3 = SH.rearrange("p (r c) -> p r c", c=PATCH)
                    nc.gpsimd.tensor_tensor(SH3[:, :, 1:31], BH[:, :, 0:30],
                                            BH[:, :, 1:31], Alu.add)
                    E2 = pool.tile([P, 32, 1], f32, tag="e2", name=f"e2{t}")
                    nc.gpsimd.tensor_tensor(E2[:], X3[:, :, 0:1],
                                            X3[:, :, 0:1], Alu.add)
                    nc.gpsimd.tensor_tensor(SH3[:, :, 0:1], BH[:, :, 0:1],
                                            E2[:], Alu.add)
                    E3 = pool.tile([P, 32, 1], f32, tag="e3", name=f"e3{t}")
                    nc.gpsimd.tensor_tensor(E3[:], X3[:, :, 31:32],
                                            X3[:, :, 31:32], Alu.add)
                    nc.gpsimd.tensor_tensor(SH3[:, :, 31:32], BH[:, :, 30:31],
                                            E3[:], Alu.add)

                    GY = pool.tile([P, HW], f32, tag=f"gy{s}", bufs=1,
                                   name=f"gy{t}")
                    nc.gpsimd.tensor_tensor(
                        GY[:, 32:992], SH[:, 64:1024], SH[:, 0:960],
                        Alu.subtract)
                    nc.gpsimd.tensor_tensor(
                        GY[:, 0:32], SH[:, 32:64], SH[:, 0:32], Alu.subtract)
                    nc.gpsimd.tensor_tensor(
                        GY[:, 992:1024], SH[:, 992:1024], SH[:, 960:992],
                        Alu.subtract)

                    if WK is not None:
                        nc.gpsimd.tensor_tensor(GX[:], GX[:], WK[:], Alu.mult)
                        nc.gpsimd.tensor_tensor(GY[:], GY[:], WK[:], Alu.mult)
                    slot[t] = [GX, GY]

                # ---- magnitude chain, op-major across the group for
                # pipeline depth (no DVE stalls on ACT/Pool latency) ----
                g2s, m0s, ms = {}, {}, {}
                for t in tiles:
                    s = t % GROUP
                    GX, GY = slot[t][0], slot[t][1]
                    # g2 = gx^2 + gy^2 + eps  (eps scaled by 8^2 vs reference)
                    G2 = pool.tile([P, HW], f32, tag=f"g2a{s}", bufs=1,
                                   name=f"g2{t}")
                    nc.vector._custom_dve(SQSUM, out=G2[:], in0=GX[:],
                                          in1=GY[:], s0=6.4e-17)
                    g2s[t] = G2
                for t in tiles:
                    s = t % GROUP
                    M0 = pool.tile([P, HW], f32, tag=f"msq{s}", bufs=1,
                                   name=f"msq{t}")
                    nc.scalar.activation(M0[:], g2s[t][:], Act.Sqrt)
                    m0s[t] = M0
                for t in tiles:
                    s = t % GROUP
                    # one Newton step: m = (1.5 - g2*rc^2*0.5)*rc*g2
                    RC = pool.tile([P, HW], f32, tag="rcf", name=f"rc{t}")
                    nc.vector.reciprocal_approx_fast(RC[:], m0s[t][:])
                    M = pool.tile([P, HW], f32, tag=f"m{s}", bufs=1,
                                  name=f"m{t}")
                    nc.vector._custom_dve(RSQRT_NRM, out=M[:], in0=g2s[t][:],
                                          in1=RC[:], s0=1.5, s1=0.5)
                    ms[t] = M

                # ---- phase B: orientation (op-major) + histogram ----
                ds, qs, a16s = {}, {}, {}
                for t in tiles:
                    s = t % GROUP
                    # d = max(m + gx, 1e-30): the clamp both avoids the
                    # recip(0)=NaN edge and pins rounding-negative d to the
                    # correct wrap side (t -> 36/0 by sign of gy).
                    D = pool.tile([P, HW], f32, tag=f"g2a{s}", bufs=1,
                                  name=f"d{t}")
                    nc.vector._custom_dve(ADDMAX, out=D[:], in0=ms[t][:],
                                          in1=slot[t][0][:], s0=1e-30)
                    ds[t] = D
                for t in tiles:
                    s = t % GROUP
                    RC = pool.tile([P, HW], f32, tag="rcf", name=f"rcb{t}")
                    SC = pool.tile([P, HW], f32, tag="scf", name=f"scb{t}")
                    nc.vector.reciprocal_approx_accurate(RC[:], ds[t][:],
                                                         SC[:])
                    Q = pool.tile([P, HW], f32, tag=f"msq{s}", bufs=1,
                                  name=f"q{t}")
                    nc.vector.scalar_tensor_tensor(
                        out=Q[:], in0=slot[t][1][:], scalar=1.0, in1=RC[:],
                        op0=Alu.mult, op1=Alu.mult)
                    qs[t] = Q
                for t in tiles:
                    s = t % GROUP
                    A = pool.tile([P, HW], f32, tag=f"a{s % 2}", bufs=2,
                                  name=f"a{t}")
                    nc.scalar.activation(A[:], qs[t][:], Act.Arctan)
                    a16s[t] = A

                for t in tiles:
                    A, M16 = a16s[t], ms[t]
                    SCR = psp.tile([P, HW], f32, tag="scr", name=f"scr{t}")
                    for k in range(NBINS):
                        acc = HEXT[:, t, k + 1:k + 2]
                        if k == 0:
                            nc.vector._custom_dve(
                                WRAP, out=SCR[:], in0=A[:], in1=M16[:],
                                s0=-17.0 * PI / 36.0, s1=17.0 * PI / 36.0,
                                accum_out=acc)
                        else:
                            nc.vector._custom_dve(
                                TRI, out=SCR[:], in0=A[:], in1=M16[:],
                                s0=c_lo(k), s1=c_hi(k), accum_out=acc)

                # ---- tail for the previous group (overlaps next group) ----
                if g > 0:
                    tail(range((g - 1) * GROUP, g * GROUP))
            tail(range((n_groups - 1) * GROUP, n_tiles))

    nc.compile()
    return nc


def _get_built(b_core, smooth_w, wk_is_ones):
    key = (b_core, tuple(float(x) for x in smooth_w), bool(wk_is_ones))
    if key not in _BUILD_CACHE:
        _BUILD_CACHE[key] = _build(b_core, smooth_w, wk_is_ones)
    return _BUILD_CACHE[key]


# --------------------------------------------------------------------------
# host entry point
# --------------------------------------------------------------------------
def kernel(patch, weight_kernel, smooth_w):
    from concourse import bass_utils

    patch = np.ascontiguousarray(np.asarray(patch, dtype=np.float32))
    weight_kernel = np.asarray(weight_kernel, dtype=np.float32)
    smooth_w = np.asarray(smooth_w, dtype=np.float32)

    B = patch.shape[0]
    assert B % (N_CORES * P) == 0, f"B={B} not divisible by {N_CORES * P}"
    b_core = B // N_CORES
    n_tiles = b_core // P

    wk_is_ones = bool(np.all(weight_kernel == 1.0))
    nc = _get_built(b_core, smooth_w, wk_is_ones)

    x = patch.reshape(N_CORES, b_core, HW)

    iota = np.tile(np.arange(NBINS, dtype=np.float32), n_tiles)
    consts_row = np.concatenate([iota, iota - 64.0]).astype(np.float32)
    consts = np.ascontiguousarray(
        np.broadcast_to(consts_row, (P, consts_row.size)))

    in_maps = []
    for i in range(N_CORES):
        m = {"patch": np.ascontiguousarray(x[i]), "consts": consts}
        if not wk_is_ones:
            m["wk"] = np.ascontiguousarray(
                np.broadcast_to(weight_kernel.reshape(-1), (P, HW)))
        in_maps.append(m)

    res = bass_utils.run_bass_kernel_spmd(nc, in_maps,
                                          core_ids=list(range(N_CORES)))
    out = np.concatenate([r["angle"] for r in res.results])
    return out.astype(np.float32)


# revision 10
# speedup vs baseline: 1.2719x; 1.0763x over previous
"""Trainium2 Bass kernel for CustomizablePatchDominantGradientOrientation.

Pipeline per patch (32x32, fp32):
  sobel (replicate pad, [1,2,1]x[-1,0,1] separable; /8 dropped - the final
  angle is invariant to a global scale on (gx, gy, mag))
  mag = sqrt(gx^2+gy^2+eps'), theta = 2*atan(gy/(mag+gx))  (half-angle atan2)
  soft 36-bin histogram of theta weighted by mag  -> 36 fused custom-DVE
  passes (relu(min(a-c0, c1-a))*mag with free-axis accumulate)
  circular [w0,w1,w2] smoothing, argmax, parabolic refinement -> angle.

Engine split: the 36 histogram passes + magnitude/orientation customs run on
DVE (the bottleneck, ~1 elem/cycle fused mask+mul+reduce); the whole sobel
runs on the otherwise-idle Pool (GPSIMD) engine as add/sub tensor_tensor
ops; sqrt/arctan run on the Scalar (ACT) engine.

Data parallel: B=32768 patches sharded over 8 NeuronCores (4096 each);
per core 32 tiles of [128 patches x 1024 pixels].  Layout is patch-major:
partitions = patches, free axis = pixels.
"""

import math

import numpy as np

NBINS = 36
PI = math.pi
PATCH = 32
HW = PATCH * PATCH
P = 128          # partitions (patches per tile)
N_CORES = 8
GROUP = 4        # tiles per ACT-table-set phase group

_BUILD_CACHE = {}
_OPS_REGISTERED = {}


# --------------------------------------------------------------------------
# custom DVE ops
# --------------------------------------------------------------------------
def _register_custom_ops():
    """Register the fused ops at runtime (row assignment + sha pin, exactly
    what a source-level `OPS.append` would do)."""
    if _OPS_REGISTERED:
        return _OPS_REGISTERED
    from operator import add as _op_add

    import concourse.dve_ops as dve_ops
    from concourse.dve_ops import DveOp
    from concourse.dve_spec import (
        Spec, Src0, Src1, C0, C1, Zero, relu, minn, maxx, lower, _has_src1,
        sq,
    )
    from concourse.dve_uop import DveOpSpec

    def _reg(name, spec):
        if name in dve_ops._SUB_OPCODE_FOR_NAME:
            for op in dve_ops.OPS:
                if op.name == name:
                    return op
        row = dve_ops._CUSTOM_DVE_ROW_BASE + len(dve_ops.OPS)
        assert row < 0x20, "custom-DVE row budget exhausted"
        dve_ops._SUB_OPCODE_FOR_NAME[name] = row
        shas = {}
        for ver in ("v3", "v4"):
            s = DveOpSpec(name=name, opcode=row, uops=lower(spec, ver=ver),
                          rd1_en=_has_src1(spec))
            shas[ver] = s.sha(ver)
        op = DveOp(name, spec, subdim=False, uops_sha=shas)
        dve_ops.OPS.append(op)
        dve_ops.CUSTOM_DVE_SPECS[name] = spec
        return op

    def _tri_ref(in0, in1, s0, s1, imm2):
        b = np.maximum(np.minimum(in0 - s0, s1 - in0), 0.0).astype(np.float32) * in1
        return b, b.reshape(b.shape[0], -1).sum(axis=-1, keepdims=True)

    def _wrap_ref(in0, in1, s0, s1, imm2):
        b = np.maximum(np.maximum(s0 - in0, in0 - s1), 0.0).astype(np.float32) * in1
        return b, b.reshape(b.shape[0], -1).sum(axis=-1, keepdims=True)

    def _sqsum_ref(in0, in1, s0, s1, imm2):
        return (in0 * in0 + in1 * in1 + s0).astype(np.float32)

    def _rsqrt_nrm_ref(in0, in1, s0, s1, imm2):
        return (((s0 - in0 * in1 * in1 * s1) * in1) * in0).astype(np.float32)

    def _addmax_ref(in0, in1, s0, s1, imm2):
        return np.maximum(in0 + in1, s0).astype(np.float32)

    _OPS_REGISTERED["tri"] = _reg(
        "HIST_TRI_ANT",
        Spec(body=relu(minn(Src0 - C0, C1 - Src0)) * Src1,
             accum=_op_add, accum_init=Zero, reference=_tri_ref))
    _OPS_REGISTERED["wrap"] = _reg(
        "HIST_WRAP_ANT",
        Spec(body=relu(maxx(C0 - Src0, Src0 - C1)) * Src1,
             accum=_op_add, accum_init=Zero, reference=_wrap_ref))
    # g2 = gx^2 + gy^2 + c0  (exact fp32 squares on DVE)
    _OPS_REGISTERED["sqsum"] = _reg(
        "SQSUM_ANT",
        Spec(body=sq(Src0) + sq(Src1) + C0, reference=_sqsum_ref))
    # m = ((c0 - g2*rc^2*c1)*rc)*g2  (Newton step toward rsqrt, times g2)
    _OPS_REGISTERED["rsqrt_nrm"] = _reg(
        "RSQRT_NRM_ANT",
        Spec(body=((C0 - Src0 * sq(Src1) * C1) * Src1) * Src0,
             reference=_rsqrt_nrm_ref))
    _OPS_REGISTERED["addmax"] = _reg(
        "ADD_MAX_ANT",
        Spec(body=maxx(Src0 + Src1, C0), reference=_addmax_ref))
    return _OPS_REGISTERED


# --------------------------------------------------------------------------
# kernel build
# --------------------------------------------------------------------------
def _build(b_core, smooth_w, wk_is_ones):
    import concourse.bacc as bacc
    import concourse.mybir as mybir
    from concourse.tile import TileContext
    from concourse.bass import broadcast_tensor_aps

    ops = _register_custom_ops()
    TRI, WRAP = ops["tri"], ops["wrap"]
    SQSUM, RSQRT_NRM, ADDMAX = ops["sqsum"], ops["rsqrt_nrm"], ops["addmax"]

    f32 = mybir.dt.float32
    f16 = mybir.dt.float16
    Alu = mybir.AluOpType
    Act = mybir.ActivationFunctionType

    n_tiles = b_core // P
    assert b_core % P == 0
    w0, w1, w2 = (float(x) for x in smooth_w)

    nc = bacc.Bacc(None, target_bir_lowering=False, debug=False)
    patch_in = nc.dram_tensor("patch", [b_core, HW], f32, kind="ExternalInput")
    # consts: iota36 repeated n_tiles times, then (iota36 - 64) repeated
    consts_in = nc.dram_tensor("consts", [P, 2 * n_tiles * NBINS], f32,
                               kind="ExternalInput")
    wk_in = None
    if not wk_is_ones:
        wk_in = nc.dram_tensor("wk", [P, HW], f32, kind="ExternalInput")
    out_t = nc.dram_tensor("angle", [b_core], f32, kind="ExternalOutput")

    # per-bin tri constants in atan units: t = (36/pi)*a + 18
    # bin k (k>=1): c0=(k-19)*pi/36, c1=(k-17)*pi/36
    # bin 0 wrap:   relu(max(c0 - a, a - c1)) with c0=-17pi/36, c1=17pi/36
    def c_lo(k):
        return (k - 19.0) * PI / 36.0

    def c_hi(k):
        return (k - 17.0) * PI / 36.0

    with TileContext(nc) as tc:
        with tc.tile_pool(name="pool", bufs=2) as pool, \
             tc.tile_pool(name="persist", bufs=1) as pp:
            IOTA = pp.tile([P, n_tiles, NBINS], f32)
            IOTA64 = pp.tile([P, n_tiles, NBINS], f32)
            nc.sync.dma_start(IOTA[:], consts_in[:, 0:n_tiles * NBINS])
            nc.sync.dma_start(IOTA64[:], consts_in[:, n_tiles * NBINS:])
            WK = None
            if wk_in is not None:
                WK = pp.tile([P, HW], f32)
                nc.sync.dma_start(WK[:], wk_in[:])

            HEXT = pp.tile([P, n_tiles, NBINS + 2], f32)
            ANG = pp.tile([P, n_tiles], f32)
            out_view = out_t[:].rearrange("(t p) -> p t", p=P)

            def tail(tiles):
                """smoothing, argmax, parabolic refinement for a tile range
                (all [P, len(tiles), ...] slices of the persistent tensors)."""
                ts = slice(tiles.start, tiles.stop)
                HX = HEXT[:, ts, :]
                nc.vector.tensor_copy(HX[:, :, 0:1], HX[:, :, 36:37])
                nc.vector.tensor_copy(HX[:, :, 37:38], HX[:, :, 1:2])

                SM = pool.tile([P, len(range(tiles.start, tiles.stop)),
                                NBINS], f32, tag="t_sm",
                               name=f"sm{tiles.start}")
                nc.vector.tensor_scalar(SM[:], HX[:, :, 2:38], w2, None,
                                        Alu.mult)
                nc.vector.scalar_tensor_tensor(
                    out=SM[:], in0=HX[:, :, 0:36], scalar=w0, in1=SM[:],
                    op0=Alu.mult, op1=Alu.add)
                HS = pool.tile([P, SM.shape[1], NBINS], f32, tag="t_hs",
                               name=f"hs{tiles.start}")
                nc.vector.scalar_tensor_tensor(
                    out=HS[:], in0=HX[:, :, 1:37], scalar=w1, in1=SM[:],
                    op0=Alu.mult, op1=Alu.add)

                IOT = IOTA[:, ts, :]
                IOT64 = IOTA64[:, ts, :]
                VMAX = pool.tile([P, SM.shape[1], 1], f32, tag="t_vm",
                                 name=f"vm{tiles.start}")
                nc.vector.tensor_reduce(VMAX[:], HS[:], mybir.AxisListType.X,
                                        Alu.max)
                EQ = pool.tile([P, SM.shape[1], NBINS], f32, tag="t_eq",
                               name=f"eq{tiles.start}")
                hs_b, vmax_b = broadcast_tensor_aps(HS[:], VMAX[:])
                nc.vector.tensor_tensor(EQ[:], hs_b, vmax_b, Alu.is_equal)
                nc.vector.tensor_tensor(EQ[:], EQ[:], IOT64, Alu.mult)
                IDX = pool.tile([P, SM.shape[1], 1], f32, tag="t_ix",
                                name=f"ix{tiles.start}")
                nc.vector.tensor_reduce(IDX[:], EQ[:], mybir.AxisListType.X,
                                        Alu.min)
                nc.vector.tensor_scalar(IDX[:], IDX[:], 64.0, None, Alu.add)

                def neighbor_value(shift, wrap_thr, wrap_add, nm):
                    IDXN = pool.tile([P, SM.shape[1], 1], f32,
                                     tag=f"t_in{nm}",
                                     name=f"idxn_{nm}{tiles.start}")
                    nc.vector.tensor_scalar(IDXN[:], IDX[:], float(shift),
                                            None, Alu.add)
                    WADJ = pool.tile([P, SM.shape[1], 1], f32,
                                     tag=f"t_wa{nm}",
                                     name=f"wadj_{nm}{tiles.start}")
                    if wrap_add < 0:
                        nc.vector.tensor_scalar(WADJ[:], IDXN[:], wrap_thr,
                                                float(wrap_add), Alu.is_gt,
                                                Alu.mult)
                    else:
                        nc.vector.tensor_scalar(WADJ[:], IDXN[:], wrap_thr,
                                                float(wrap_add), Alu.is_lt,
                                                Alu.mult)
                    nc.vector.tensor_tensor(IDXN[:], IDXN[:], WADJ[:],
                                            Alu.add)
                    DIF = pool.tile([P, SM.shape[1], NBINS], f32,
                                    tag=f"t_df{nm}",
                                    name=f"dif_{nm}{tiles.start}")
                    iota_b, idxn_b = broadcast_tensor_aps(IOT, IDXN[:])
                    nc.vector.tensor_tensor(DIF[:], iota_b, idxn_b,
                                            Alu.subtract)
                    nc.vector.tensor_scalar(DIF[:], DIF[:], 0.0, None,
                                            Alu.is_equal)
                    nc.vector.tensor_tensor(DIF[:], DIF[:], HS[:], Alu.mult)
                    V = pool.tile([P, SM.shape[1], 1], f32, tag=f"t_v{nm}",
                                  name=f"v_{nm}{tiles.start}")
                    nc.vector.tensor_reduce(V[:], DIF[:],
                                            mybir.AxisListType.X, Alu.add)
                    return V

                VP = neighbor_value(+1, 35.5, -36.0, "p")
                VM = neighbor_value(-1, -0.5, +36.0, "m")

                NUM = pool.tile([P, SM.shape[1], 1], f32, tag="t_nu",
                                name=f"nu{tiles.start}")
                nc.vector.tensor_tensor(NUM[:], VP[:], VM[:], Alu.subtract)
                SUMN = pool.tile([P, SM.shape[1], 1], f32, tag="t_su",
                                 name=f"su{tiles.start}")
                nc.vector.tensor_tensor(SUMN[:], VP[:], VM[:], Alu.add)
                DEN = pool.tile([P, SM.shape[1], 1], f32, tag="t_de",
                                name=f"de{tiles.start}")
                nc.vector.tensor_scalar(DEN[:], VMAX[:], 2.0, None, Alu.mult)
                nc.vector.tensor_tensor(DEN[:], DEN[:], SUMN[:], Alu.subtract)
                RECD = pool.tile([P, SM.shape[1], 1], f32, tag="t_rd",
                                 name=f"rd{tiles.start}")
                SCD = pool.tile([P, SM.shape[1], 1], f32, tag="t_sc",
                                name=f"sc{tiles.start}")
                nc.vector.reciprocal_approx_accurate(RECD[:], DEN[:], SCD[:])
                REF = pool.tile([P, SM.shape[1], 1], f32, tag="t_rf",
                                name=f"rf{tiles.start}")
                nc.vector.scalar_tensor_tensor(
                    out=REF[:], in0=NUM[:], scalar=0.5, in1=RECD[:],
                    op0=Alu.mult, op1=Alu.mult)
                nc.vector.tensor_tensor(REF[:], IDX[:], REF[:], Alu.add)
                nc.vector.tensor_scalar(ANG[:, ts], REF[:, :, 0],
                                        -2.0 * PI / NBINS, PI, Alu.mult,
                                        Alu.add)
                nc.sync.dma_start(out_view[:, ts], ANG[:, ts])

            n_groups = (n_tiles + GROUP - 1) // GROUP
            for g in range(n_groups):
                tiles = range(g * GROUP, min((g + 1) * GROUP, n_tiles))
                slot = {}
                # ---- phase A: sobel (Pool), magnitude (DVE + sqrt table) --
                for t in tiles:
                    s = t % GROUP
                    X = pool.tile([P, HW], f32, tag="x", bufs=3, name=f"x{t}")
                    nc.sync.dma_start(X[:], patch_in[t * P:(t + 1) * P, :])
                    X3 = X.rearrange("p (r c) -> p r c", c=PATCH)

                    SV = pool.tile([P, HW], f32, tag="sv", name=f"sv{t}")
                    # vertical [1,2,1] with replicate rows
                    nc.vector.scalar_tensor_tensor(
                        out=SV[:, 32:992], in0=X[:, 32:992], scalar=2.0,
                        in1=X[:, 0:960], op0=Alu.mult, op1=Alu.add)
                    nc.vector.tensor_tensor(
                        SV[:, 32:992], SV[:, 32:992], X[:, 64:1024], Alu.add)
                    nc.vector.scalar_tensor_tensor(
                        out=SV[:, 0:32], in0=X[:, 0:32], scalar=3.0,
                        in1=X[:, 32:64], op0=Alu.mult, op1=Alu.add)
                    nc.vector.scalar_tensor_tensor(
                        out=SV[:, 992:1024], in0=X[:, 992:1024], scalar=3.0,
                        in1=X[:, 960:992], op0=Alu.mult, op1=Alu.add)
                    SV3 = SV.rearrange("p (r c) -> p r c", c=PATCH)

                    GX = pool.tile([P, HW], f32, tag=f"gx{s}", bufs=1,
                                   name=f"gx{t}")
                    GX3 = GX.rearrange("p (r c) -> p r c", c=PATCH)
                    # horizontal central difference with replicate cols
                    nc.vector.tensor_tensor(
                        GX3[:, :, 1:31], SV3[:, :, 2:32], SV3[:, :, 0:30],
                        Alu.subtract)
                    nc.vector.tensor_tensor(
                        GX3[:, :, 0:1], SV3[:, :, 1:2], SV3[:, :, 0:1],
                        Alu.subtract)
                    nc.vector.tensor_tensor(
                        GX3[:, :, 31:32], SV3[:, :, 31:32], SV3[:, :, 30:31],
                        Alu.subtract)

                    SH = pool.tile([P, HW], f32, tag="sh", name=f"sh{t}")
                    SH3 = SH.rearrange("p (r c) -> p r c", c=PATCH)
                    # horizontal [1,2,1] with replicate cols
                    nc.vector.scalar_tensor_tensor(
                        out=SH3[:, :, 1:31], in0=X3[:, :, 1:31], scalar=2.0,
                        in1=X3[:, :, 0:30], op0=Alu.mult, op1=Alu.add)
                    nc.vector.tensor_tensor(
                        SH3[:, :, 1:31], SH3[:, :, 1:31], X3[:, :, 2:32],
                        Alu.add)
                    nc.vector.scalar_tensor_tensor(
                        out=SH3[:, :, 0:1], in0=X3[:, :, 0:1], scalar=3.0,
                        in1=X3[:, :, 1:2], op0=Alu.mult, op1=Alu.add)
                    nc.vector.scalar_tensor_tensor(
                        out=SH3[:, :, 31:32], in0=X3[:, :, 31:32], scalar=3.0,
                        in1=X3[:, :, 30:31], op0=Alu.mult, op1=Alu.add)

                    GY = pool.tile([P, HW], f32, tag=f"gy{s}", bufs=1,
                                   name=f"gy{t}")
                    # vertical central difference with replicate rows
                    nc.vector.tensor_tensor(
                        GY[:, 32:992], SH[:, 64:1024], SH[:, 0:960],
                        Alu.subtract)
                    nc.vector.tensor_tensor(
                        GY[:, 0:32], SH[:, 32:64], SH[:, 0:32], Alu.subtract)
                    nc.vector.tensor_tensor(
                        GY[:, 992:1024], SH[:, 992:1024], SH[:, 960:992],
                        Alu.subtract)

                    if WK is not None:
                        nc.vector.tensor_tensor(GX[:], GX[:], WK[:], Alu.mult)
                        nc.vector.tensor_tensor(GY[:], GY[:], WK[:], Alu.mult)
                    slot[t] = [GX, GY]

                # ---- magnitude chain, op-major across the group for
                # pipeline depth (no DVE stalls on ACT/Pool latency) ----
                g2s, m0s, ms = {}, {}, {}
                for t in tiles:
                    s = t % GROUP
                    GX, GY = slot[t][0], slot[t][1]
                    # g2 = gx^2 + gy^2 + eps  (eps scaled by 8^2 vs reference)
                    G2 = pool.tile([P, HW], f32, tag=f"g2a{s}", bufs=1,
                                   name=f"g2{t}")
                    nc.vector._custom_dve(SQSUM, out=G2[:], in0=GX[:],
                                          in1=GY[:], s0=6.4e-17)
                    g2s[t] = G2
                for t in tiles:
                    s = t % GROUP
                    M0 = pool.tile([P, HW], f32, tag=f"msq{s}", bufs=1,
                                   name=f"msq{t}")
                    nc.scalar.activation(M0[:], g2s[t][:], Act.Sqrt)
                    m0s[t] = M0
                for t in tiles:
                    s = t % GROUP
                    # one Newton step: m = (1.5 - g2*rc^2*0.5)*rc*g2
                    RC = pool.tile([P, HW], f32, tag="rcf", name=f"rc{t}")
                    nc.vector.reciprocal_approx_fast(RC[:], m0s[t][:])
                    M = pool.tile([P, HW], f32, tag=f"m{s}", bufs=1,
                                  name=f"m{t}")
                    nc.vector._custom_dve(RSQRT_NRM, out=M[:], in0=g2s[t][:],
                                          in1=RC[:], s0=1.5, s1=0.5)
                    ms[t] = M

                # ---- phase B: orientation (op-major) + histogram ----
                ds, qs, a16s = {}, {}, {}
                for t in tiles:
                    s = t % GROUP
                    # d = max(m + gx, 1e-30): the clamp both avoids the
                    # recip(0)=NaN edge and pins rounding-negative d to the
                    # correct wrap side (t -> 36/0 by sign of gy).
                    D = pool.tile([P, HW], f32, tag=f"g2a{s}", bufs=1,
                                  name=f"d{t}")
                    nc.vector._custom_dve(ADDMAX, out=D[:], in0=ms[t][:],
                                          in1=slot[t][0][:], s0=1e-30)
                    ds[t] = D
                for t in tiles:
                    s = t % GROUP
                    RC = pool.tile([P, HW], f32, tag="rcf", name=f"rcb{t}")
                    SC = pool.tile([P, HW], f32, tag="scf", name=f"scb{t}")
                    nc.vector.reciprocal_approx_accurate(RC[:], ds[t][:],
                                                         SC[:])
                    Q = pool.tile([P, HW], f32, tag=f"msq{s}", bufs=1,
                                  name=f"q{t}")
                    nc.vector.scalar_tensor_tensor(
                        out=Q[:], in0=slot[t][1][:], scalar=1.0, in1=RC[:],
                        op0=Alu.mult, op1=Alu.mult)
                    qs[t] = Q
                for t in tiles:
                    s = t % GROUP
                    A = pool.tile([P, HW], f32, tag=f"a{s % 2}", bufs=2,
                                  name=f"a{t}")
                    nc.scalar.activation(A[:], qs[t][:], Act.Arctan)
                    a16s[t] = A

                for t in tiles:
                    A, M16 = a16s[t], ms[t]
                    SCR = pool.tile([P, HW], f32, tag="scr", name=f"scr{t}")
                    for k in range(NBINS):
                        acc = HEXT[:, t, k + 1:k + 2]
                        if k == 0:
                            nc.vector._custom_dve(
                                WRAP, out=SCR[:], in0=A[:], in1=M16[:],
                                s0=-17.0 * PI / 36.0, s1=17.0 * PI / 36.0,
                                accum_out=acc)
                        else:
                            nc.vector._custom_dve(
                                TRI, out=SCR[:], in0=A[:], in1=M16[:],
                                s0=c_lo(k), s1=c_hi(k), accum_out=acc)

                # ---- tail for the previous group (overlaps next group) ----
                if g > 0:
                    tail(range((g - 1) * GROUP, g * GROUP))
            tail(range((n_groups - 1) * GROUP, n_tiles))

    nc.compile()
    return nc


def _get_built(b_core, smooth_w, wk_is_ones):
    key = (b_core, tuple(float(x) for x in smooth_w), bool(wk_is_ones))
    if key not in _BUILD_CACHE:
        _BUILD_CACHE[key] = _build(b_core, smooth_w, wk_is_ones)
    return _BUILD_CACHE[key]


# --------------------------------------------------------------------------
# host entry point
# --------------------------------------------------------------------------
def kernel(patch, weight_kernel, smooth_w):
    from concourse import bass_utils

    patch = np.ascontiguousarray(np.asarray(patch, dtype=np.float32))
    weight_kernel = np.asarray(weight_kernel, dtype=np.float32)
    smooth_w = np.asarray(smooth_w, dtype=np.float32)

    B = patch.shape[0]
    assert B % (N_CORES * P) == 0, f"B={B} not divisible by {N_CORES * P}"
    b_core = B // N_CORES
    n_tiles = b_core // P

    wk_is_ones = bool(np.all(weight_kernel == 1.0))
    nc = _get_built(b_core, smooth_w, wk_is_ones)

    x = patch.reshape(N_CORES, b_core, HW)

    iota = np.tile(np.arange(NBINS, dtype=np.float32), n_tiles)
    consts_row = np.concatenate([iota, iota - 64.0]).astype(np.float32)
    consts = np.ascontiguousarray(
        np.broadcast_to(consts_row, (P, consts_row.size)))

    in_maps = []
    for i in range(N_CORES):
        m = {"patch": np.ascontiguousarray(x[i]), "consts": consts}
        if not wk_is_ones:
            m["wk"] = np.ascontiguousarray(
                np.broadcast_to(weight_kernel.reshape(-1), (P, HW)))
        in_maps.append(m)

    res = bass_utils.run_bass_kernel_spmd(nc, in_maps,
                                          core_ids=list(range(N_CORES)))
    out = np.concatenate([r["angle"] for r in res.results])
    return out.astype(np.float32)
